# revision 8
# baseline (speedup 1.0000x reference)
"""Trainium2 Bass kernel for DifferentiableSoftmaxTree NLL (hierarchical
softmax negative log-likelihood).

Math: the 2-way log_softmax at each tree node reduces to a softplus of a
signed logit difference: for sample b with path nodes n_k / directions d_k,
    s_k  = features[b] . (node_weights[n_k,:,1] - node_weights[n_k,:,0])
    out[b] = sum_k mask_k * softplus((1-2 d_k) * s_k)

Strategy (data-parallel over batch, 8 cores x 512 samples), exploiting
that the top tree levels are SHARED across samples:

  * Levels 0..8 (nodes 0..510, on every path): computed DENSELY on the
    TensorEngine. Per 128-sample block, accumulating bf16 matmuls
    (contraction over feature chunks of 128) produce U[sample, node] for
    all top nodes in PSUM. Column layout pads levels 0..6 to 64 columns
    each so the per-sample +-1 one-hot selection is one product plus
    THREE tensor_reduce ops ([128,7,64] / level-7 / level-8) instead of
    nine overhead-dominated ones.
  * Levels 9..15 (near-distinct nodes): per-class contiguous bf16 gather
    of sgn*mask-folded diff rows (3.5KB/sample), one single-offset SWDGE
    indirect DMA per 128-sample block, elementwise product against the
    feature row (split DVE/GpSimd), one [128,7,512] tensor_reduce.
  * Missing level 15 (15-edge paths) is a -30 bias column: softplus(-30)~=0.
  * softplus: per block only Exp (sharing the ACT table with Identity);
    ln(1+e) is ONE Ln op (bias=1.0) over all 4 blocks' stashed exponentials
    at the end + one 16-wide product reduce -> 2 ACT table loads total
    instead of 8 (table thrash cost 10.3us in the v3 trace).

All tables/masks are bf16 (rel err ~2e-3 vs the 2e-2 gate). Column
layouts keep every DVE operand 4-byte aligned for 2x packed mode.
"""

import numpy as np
from contextlib import ExitStack

import concourse.bass as bass
import concourse.mybir as mybir
import concourse.tile as tile
from concourse import bass_utils
import concourse.bacc as bacc
import ml_dtypes

NUM_CLASSES = 50000
NUM_INTERNAL = NUM_CLASSES - 1
D = 512
B = 4096
K = 16
TOPL = 9                   # levels via dense matmul
DEEPL = K - TOPL           # 7 levels via gather
N_CORES = 8
BL = B // N_CORES          # samples per core
P = 128                    # partition dim
NBLK = BL // P             # 128-sample blocks per core
NCH = D // P               # feature chunks for matmul contraction

# top-level column layout: levels 0..6 padded to 64 cols each (448), then
# 64 zero cols (so region A is a full 512-col PSUM bank), then level 7
# (128 cols) and level 8 (256 cols) as region B.
WA = 512                   # region A width (7*64 used + 64 zero)
WB = 384                   # region B width (128 + 256)
WTOT = WA + WB             # 896 columns per chunk
L7OFF = WA                 # level-7 cols [512, 640)
L8OFF = WA + 128           # level-8 cols [640, 896)
DVE_DEEP = 5               # deep product levels on DVE (rest on GpSimd)

BF16 = ml_dtypes.bfloat16

_AF = mybir.ActivationFunctionType
_OP = mybir.AluOpType


def _build_program():
    nc = bacc.Bacc(
        "TRN2",
        target_bir_lowering=False,
        debug=False,
        enable_asserts=False,
        num_devices=N_CORES,
    )
    ptabd_ap = nc.dram_tensor(
        "ptabd", [NUM_CLASSES, DEEPL * D], mybir.dt.bfloat16, kind="ExternalInput"
    ).ap()
    idx_ap = nc.dram_tensor("idx", [P, NBLK], mybir.dt.int32, kind="ExternalInput").ap()
    featb_ap = nc.dram_tensor(
        "featb", [P, NBLK * D], mybir.dt.bfloat16, kind="ExternalInput"
    ).ap()
    featT_ap = nc.dram_tensor(
        "featT", [P, NCH * BL], mybir.dt.bfloat16, kind="ExternalInput"
    ).ap()
    dtopT_ap = nc.dram_tensor(
        "dtopT", [P, NCH * WTOT], mybir.dt.bfloat16, kind="ExternalInput"
    ).ap()
    meta_ap = nc.dram_tensor(
        "meta", [BL, WTOT + K], mybir.dt.bfloat16, kind="ExternalInput"
    ).ap()
    out_ap = nc.dram_tensor("out", [BL, 1], mybir.dt.float32, kind="ExternalOutput").ap()

    with tile.TileContext(nc) as tc, ExitStack() as ctx:
        once_pool = ctx.enter_context(tc.tile_pool(name="once", bufs=1))
        meta_pool = ctx.enter_context(tc.tile_pool(name="meta", bufs=2))
        gath_pool = ctx.enter_context(tc.tile_pool(name="gath", bufs=3))
        ub_pool = ctx.enter_context(tc.tile_pool(name="ub", bufs=2))
        scr_pool = ctx.enter_context(tc.tile_pool(name="scr", bufs=2))
        sel_pool = ctx.enter_context(tc.tile_pool(name="sel", bufs=2))
        psum_pool = ctx.enter_context(tc.tile_pool(name="psum", bufs=2, space="PSUM"))

        idx_t = once_pool.tile([P, NBLK], mybir.dt.int32, tag="idx")
        nc.sync.dma_start(idx_t[:], idx_ap[:])
        featb_t = once_pool.tile([P, NBLK * D], mybir.dt.bfloat16, tag="featb")
        nc.sync.dma_start(featb_t[:], featb_ap[:])
        featT_t = once_pool.tile([P, NCH * BL], mybir.dt.bfloat16, tag="featT")
        nc.sync.dma_start(featT_t[:], featT_ap[:])
        dtopT_t = once_pool.tile([P, NCH * WTOT], mybir.dt.bfloat16, tag="dtopT")
        nc.sync.dma_start(dtopT_t[:], dtopT_ap[:])
        # e^sel stash for all blocks; ln(1+e) + per-block sum happen once at end
        eall_t = once_pool.tile([P, NBLK * K], mybir.dt.float32, tag="eall")

        for blk in range(NBLK):
            b0 = blk * P
            meta_t = meta_pool.tile([P, WTOT + K], mybir.dt.bfloat16, tag="meta")
            nc.sync.dma_start(meta_t[:], meta_ap[b0 : b0 + P, :])

            # deep-path gather: one contiguous 3.5KB row per sample
            g_t = gath_pool.tile([P, DEEPL * D], mybir.dt.bfloat16, tag="g")
            nc.gpsimd.indirect_dma_start(
                out=g_t[:],
                out_offset=None,
                in_=ptabd_ap[:],
                in_offset=bass.IndirectOffsetOnAxis(ap=idx_t[:, blk : blk + 1], axis=0),
            )

            # top levels: U[sample, col] = sum_d feat[sample, d]*difftop[col, d]
            # region A (bank 0) then region B (bank 1) of one PSUM tile
            u_t = psum_pool.tile([P, 2 * WA], mybir.dt.float32, tag="u")
            for c in range(NCH):
                nc.tensor.matmul(
                    u_t[:, 0:WA],
                    lhsT=featT_t[:, c * BL + b0 : c * BL + b0 + P],
                    rhs=dtopT_t[:, c * WTOT : c * WTOT + WA],
                    start=(c == 0),
                    stop=(c == NCH - 1),
                )
            for c in range(NCH):
                nc.tensor.matmul(
                    u_t[:, WA : WA + WB],
                    lhsT=featT_t[:, c * BL + b0 : c * BL + b0 + P],
                    rhs=dtopT_t[:, c * WTOT + WA : (c + 1) * WTOT],
                    start=(c == 0),
                    stop=(c == NCH - 1),
                )
            ub_t = ub_pool.tile([P, WTOT], mybir.dt.bfloat16, tag="ub")
            nc.scalar.activation(ub_t[:], u_t[:, 0:WTOT], _AF.Identity)

            sel_t = sel_pool.tile([P, K], mybir.dt.float32, tag="sel")
            # top selection product on GpSimd, per-level reduces on DVE
            prT_t = ub_pool.tile([P, WTOT], mybir.dt.bfloat16, tag="prT")
            nc.gpsimd.tensor_tensor(
                out=prT_t[:], in0=ub_t[:], in1=meta_t[:, 0:WTOT], op=_OP.mult
            )
            nc.vector.tensor_reduce(
                out=sel_t[:, 0:7],
                in_=prT_t[:, 0 : 7 * 64].rearrange("p (k d) -> p k d", k=7),
                axis=mybir.AxisListType.X,
                op=_OP.add,
            )
            nc.vector.tensor_reduce(
                out=sel_t[:, 7:8],
                in_=prT_t[:, L7OFF : L7OFF + 128],
                axis=mybir.AxisListType.X,
                op=_OP.add,
            )
            nc.vector.tensor_reduce(
                out=sel_t[:, 8:9],
                in_=prT_t[:, L8OFF : L8OFF + 256],
                axis=mybir.AxisListType.X,
                op=_OP.add,
            )

            # deep product: levels 0..4 on DVE, 5..6 on GpSimd, one reduce
            pr_t = scr_pool.tile([P, DEEPL * D], mybir.dt.bfloat16, tag="pr")
            fview = featb_t[:, blk * D : (blk + 1) * D]
            nc.vector.tensor_tensor(
                out=pr_t[:, 0 : DVE_DEEP * D].rearrange("p (k d) -> p k d", k=DVE_DEEP),
                in0=g_t[:, 0 : DVE_DEEP * D].rearrange("p (k d) -> p k d", k=DVE_DEEP),
                in1=fview[:, None, :].to_broadcast([P, DVE_DEEP, D]),
                op=_OP.mult,
            )
            nc.gpsimd.tensor_tensor(
                out=pr_t[:, DVE_DEEP * D :].rearrange(
                    "p (k d) -> p k d", k=DEEPL - DVE_DEEP
                ),
                in0=g_t[:, DVE_DEEP * D :].rearrange(
                    "p (k d) -> p k d", k=DEEPL - DVE_DEEP
                ),
                in1=fview[:, None, :].to_broadcast([P, DEEPL - DVE_DEEP, D]),
                op=_OP.mult,
            )
            nc.vector.tensor_reduce(
                out=sel_t[:, TOPL:K],
                in_=pr_t[:].rearrange("p (k d) -> p k d", k=DEEPL),
                axis=mybir.AxisListType.X,
                op=_OP.add,
            )
            # masked-level-15 bias: sel += biasK (0 or -30)
            nc.gpsimd.tensor_tensor(
                out=sel_t[:],
                in0=sel_t[:],
                in1=meta_t[:, WTOT : WTOT + K],
                op=_OP.add,
            )
            # stash e^sel; ln/sum deferred past the block loop
            nc.scalar.activation(
                eall_t[:, blk * K : (blk + 1) * K], sel_t[:], _AF.Exp
            )

        # out[b] = sum_k ln(1 + e^sel) : one Ln (bias=1) + one 16-wide reduce
        spall_t = once_pool.tile([P, NBLK * K], mybir.dt.float32, tag="spall")
        nc.scalar.activation(spall_t[:], eall_t[:], _AF.Ln, bias=1.0)
        lnres_t = once_pool.tile([P, NBLK], mybir.dt.float32, tag="lnres")
        nc.vector.tensor_reduce(
            out=lnres_t[:],
            in_=spall_t[:].rearrange("p (blk k) -> p blk k", blk=NBLK),
            axis=mybir.AxisListType.X,
            op=_OP.add,
        )
        for blk in range(NBLK):
            nc.sync.dma_start(
                out_ap[blk * P : (blk + 1) * P, :], lnres_t[:, blk : blk + 1]
            )

    nc.compile()
    return nc


_PROGRAM_CACHE = {}


def _get_program():
    if "nc" not in _PROGRAM_CACHE:
        _PROGRAM_CACHE["nc"] = _build_program()
    return _PROGRAM_CACHE["nc"]


def _reset_device():
    # A previously-crashed kernel can leave an exec unit wedged; a
    # client-side axon reset clears it and is near-free otherwise.
    try:
        import ctypes

        lib = ctypes.CDLL("/opt/axon/libaxon_pjrt.so")
        lib.axon_reset.restype = ctypes.c_int64
        lib.axon_reset()
    except Exception:
        pass


def _prepare_inputs(features, targets, node_weights, path_nodes_map, path_directions_map):
    feat = np.asarray(features, dtype=np.float32)
    t = np.asarray(targets, dtype=np.int32).reshape(-1)
    nw = np.asarray(node_weights, dtype=np.float32)
    pn = np.asarray(path_nodes_map, dtype=np.int32)
    pd = np.asarray(path_directions_map, dtype=np.int32)

    diff = nw[:, :, 1] - nw[:, :, 0]                        # [N_INT, D]

    # deep per-class table, levels 9..15, sign+mask folded, bf16
    nodes_d = pn[:, TOPL:]
    dirs_d = pd[:, TOPL:]
    maskd = nodes_d != -1
    safed = np.where(maskd, nodes_d, 0)
    sgnd = np.where(maskd, 1 - 2 * dirs_d, 0).astype(np.float32)
    ptabd = (diff[safed] * sgnd[:, :, None]).reshape(NUM_CLASSES, DEEPL * D)
    ptabd = np.ascontiguousarray(ptabd.astype(BF16))

    # top-node column layout (within each feature chunk):
    #   level j<7 -> cols j*64 + (node - (2^j-1));  cols 448..511 zero
    #   level 7   -> cols 512 + (node-127);  level 8 -> cols 640 + (node-255)
    topcol = np.zeros(2 ** TOPL - 1, np.int64)
    for j in range(7):
        lo = 2 ** j - 1
        topcol[lo : 2 * lo + 1] = j * 64 + np.arange(lo + 1)
    topcol[127:255] = L7OFF + np.arange(128)
    topcol[255:511] = L8OFF + np.arange(256)

    dtopT = np.zeros((P, NCH, WTOT), np.float32)
    dtopT[:, :, topcol] = diff[: 2 ** TOPL - 1].reshape(-1, NCH, P).transpose(2, 1, 0)
    dtopT = np.ascontiguousarray(dtopT.reshape(P, NCH * WTOT).astype(BF16))

    # per-sample meta: +-1 one-hot over padded top cols | biasK
    n9 = pn[t, :TOPL]                                       # [B, 9] all valid
    d9 = pd[t, :TOPL]
    oh = np.zeros((B, WTOT), np.float32)
    oh[np.arange(B)[:, None], topcol[n9]] = 1 - 2 * d9
    biasK = np.zeros((B, K), np.float32)
    biasK[:, K - 1] = np.where(pn[t, K - 1] == -1, -30.0, 0.0)
    meta = np.ascontiguousarray(
        np.concatenate([oh, biasK], axis=1).astype(BF16)     # [B, WTOT+K]
    )

    per_core = []
    for i in range(N_CORES):
        sl = slice(i * BL, (i + 1) * BL)
        fc = feat[sl]
        tc_ = t[sl]
        featb = np.ascontiguousarray(
            fc.reshape(NBLK, P, D).transpose(1, 0, 2).reshape(P, NBLK * D).astype(BF16)
        )
        featT = np.ascontiguousarray(
            fc.reshape(BL, NCH, P).transpose(2, 1, 0).reshape(P, NCH * BL).astype(BF16)
        )
        idx = np.ascontiguousarray(tc_.reshape(NBLK, P).T.astype(np.int32))
        per_core.append(
            {
                "ptabd": ptabd,
                "dtopT": dtopT,
                "idx": idx,
                "featb": featb,
                "featT": featT,
                "meta": meta[sl],
            }
        )
    return per_core


def kernel(features, targets, node_weights, path_nodes_map, path_directions_map):
    in_maps = _prepare_inputs(
        features, targets, node_weights, path_nodes_map, path_directions_map
    )
    _reset_device()
    nc = _get_program()
    res = bass_utils.run_bass_kernel_spmd(nc, in_maps, core_ids=list(range(N_CORES)))
    out = np.concatenate([res.results[i]["out"].reshape(-1) for i in range(N_CORES)])
    return out.astype(np.float32)


# revision 9
# speedup vs baseline: 1.1839x; 1.1839x over previous
"""Trainium2 Bass kernel for DifferentiableSoftmaxTree NLL (hierarchical
softmax negative log-likelihood).

Math: the 2-way log_softmax at each tree node reduces to a softplus of a
signed logit difference: for sample b with path nodes n_k / directions d_k,
    s_k  = features[b] . (node_weights[n_k,:,1] - node_weights[n_k,:,0])
    out[b] = sum_k mask_k * softplus((1-2 d_k) * s_k)

Strategy (data-parallel over batch, 8 cores x 512 samples), exploiting
that the top tree levels are SHARED across samples:

  * Levels 0..8 (nodes 0..510, on every path): computed DENSELY on the
    TensorEngine. Per 128-sample block, accumulating bf16 matmuls
    (contraction over feature chunks of 128) produce U[sample, node] for
    all top nodes in PSUM. Column layout pads levels 0..6 to 64 columns
    each so the per-sample +-1 one-hot selection is one product plus
    THREE tensor_reduce ops ([128,7,64] / level-7 / level-8) instead of
    nine overhead-dominated ones.
  * Levels 9..15 (near-distinct nodes): per-class contiguous bf16 gather
    of sgn*mask-folded diff rows (3.5KB/sample), one single-offset SWDGE
    indirect DMA per 128-sample block, elementwise product against the
    feature row (split DVE/GpSimd), one [128,7,512] tensor_reduce.
  * Missing level 15 (15-edge paths) is a -30 bias column: softplus(-30)~=0.
  * softplus: per block only Exp (sharing the ACT table with Identity);
    ln(1+e) is ONE Ln op (bias=1.0) over all 4 blocks' stashed exponentials
    at the end + one 16-wide product reduce -> 2 ACT table loads total
    instead of 8 (table thrash cost 10.3us in the v3 trace).

All tables/masks are bf16 (rel err ~2e-3 vs the 2e-2 gate). Column
layouts keep every DVE operand 4-byte aligned for 2x packed mode.
"""

import numpy as np
from contextlib import ExitStack

import concourse.bass as bass
import concourse.mybir as mybir
import concourse.tile as tile
from concourse import bass_utils
import concourse.bacc as bacc
import ml_dtypes

NUM_CLASSES = 50000
NUM_INTERNAL = NUM_CLASSES - 1
D = 512
B = 4096
K = 16
TOPL = 9                   # levels via dense matmul
DEEPL = K - TOPL           # 7 levels via gather
N_CORES = 8
BL = B // N_CORES          # samples per core
P = 128                    # partition dim
NBLK = BL // P             # 128-sample blocks per core
NCH = D // P               # feature chunks for matmul contraction

# top-level column layout: levels 0..6 padded to 64 cols each (448), then
# 64 zero cols (so region A is a full 512-col PSUM bank), then level 7
# (128 cols) and level 8 (256 cols) as region B.
WA = 512                   # region A width (7*64 used + 64 zero)
WB = 384                   # region B width (128 + 256)
WTOT = WA + WB             # 896 columns per chunk
L7OFF = WA                 # level-7 cols [512, 640)
L8OFF = WA + 128           # level-8 cols [640, 896)
DVE_DEEP = 5               # deep product levels on DVE (rest on GpSimd)

BF16 = ml_dtypes.bfloat16

_AF = mybir.ActivationFunctionType
_OP = mybir.AluOpType


def _build_program():
    nc = bacc.Bacc(
        "TRN2",
        target_bir_lowering=False,
        debug=False,
        enable_asserts=False,
        num_devices=N_CORES,
    )
    ptabd_ap = nc.dram_tensor(
        "ptabd", [NUM_CLASSES, DEEPL * D], mybir.dt.bfloat16, kind="ExternalInput"
    ).ap()
    idx_ap = nc.dram_tensor("idx", [P, NBLK], mybir.dt.int32, kind="ExternalInput").ap()
    featb_ap = nc.dram_tensor(
        "featb", [P, NBLK * D], mybir.dt.bfloat16, kind="ExternalInput"
    ).ap()
    featT_ap = nc.dram_tensor(
        "featT", [P, NCH * BL], mybir.dt.bfloat16, kind="ExternalInput"
    ).ap()
    dtopT_ap = nc.dram_tensor(
        "dtopT", [P, NCH * WTOT], mybir.dt.bfloat16, kind="ExternalInput"
    ).ap()
    meta_ap = nc.dram_tensor(
        "meta", [BL, WTOT + K], mybir.dt.bfloat16, kind="ExternalInput"
    ).ap()
    out_ap = nc.dram_tensor("out", [BL, 1], mybir.dt.float32, kind="ExternalOutput").ap()

    with tile.TileContext(nc) as tc, ExitStack() as ctx:
        once_pool = ctx.enter_context(tc.tile_pool(name="once", bufs=1))
        meta_pool = ctx.enter_context(tc.tile_pool(name="meta", bufs=2))
        gath_pool = ctx.enter_context(tc.tile_pool(name="gath", bufs=3))
        ub_pool = ctx.enter_context(tc.tile_pool(name="ub", bufs=2))
        scr_pool = ctx.enter_context(tc.tile_pool(name="scr", bufs=2))
        sel_pool = ctx.enter_context(tc.tile_pool(name="sel", bufs=2))
        psum_pool = ctx.enter_context(tc.tile_pool(name="psum", bufs=2, space="PSUM"))

        idx_t = once_pool.tile([P, NBLK], mybir.dt.int32, tag="idx")
        nc.sync.dma_start(idx_t[:], idx_ap[:])
        featb_t = once_pool.tile([P, NBLK * D], mybir.dt.bfloat16, tag="featb")
        nc.sync.dma_start(featb_t[:], featb_ap[:])
        featT_t = once_pool.tile([P, NCH * BL], mybir.dt.bfloat16, tag="featT")
        nc.sync.dma_start(featT_t[:], featT_ap[:])
        dtopT_t = once_pool.tile([P, NCH * WTOT], mybir.dt.bfloat16, tag="dtopT")
        nc.sync.dma_start(dtopT_t[:], dtopT_ap[:])
        # e^sel stash for all blocks; ln(1+e) + per-block sum happen once at end
        eall_t = once_pool.tile([P, NBLK * K], mybir.dt.float32, tag="eall")

        for blk in range(NBLK):
            b0 = blk * P
            meta_t = meta_pool.tile([P, WTOT + K], mybir.dt.bfloat16, tag="meta")
            nc.sync.dma_start(meta_t[:], meta_ap[b0 : b0 + P, :])

            # deep-path gather: one contiguous 3.5KB row per sample
            g_t = gath_pool.tile([P, DEEPL * D], mybir.dt.bfloat16, tag="g")
            nc.gpsimd.indirect_dma_start(
                out=g_t[:],
                out_offset=None,
                in_=ptabd_ap[:],
                in_offset=bass.IndirectOffsetOnAxis(ap=idx_t[:, blk : blk + 1], axis=0),
            )

            # top levels: U[sample, col] = sum_d feat[sample, d]*difftop[col, d]
            # region A (bank 0) then region B (bank 1) of one PSUM tile
            u_t = psum_pool.tile([P, 2 * WA], mybir.dt.float32, tag="u")
            for c in range(NCH):
                nc.tensor.matmul(
                    u_t[:, 0:WA],
                    lhsT=featT_t[:, c * BL + b0 : c * BL + b0 + P],
                    rhs=dtopT_t[:, c * WTOT : c * WTOT + WA],
                    start=(c == 0),
                    stop=(c == NCH - 1),
                )
            for c in range(NCH):
                nc.tensor.matmul(
                    u_t[:, WA : WA + WB],
                    lhsT=featT_t[:, c * BL + b0 : c * BL + b0 + P],
                    rhs=dtopT_t[:, c * WTOT + WA : (c + 1) * WTOT],
                    start=(c == 0),
                    stop=(c == NCH - 1),
                )
            ub_t = ub_pool.tile([P, WTOT], mybir.dt.bfloat16, tag="ub")
            nc.scalar.activation(ub_t[:], u_t[:, 0:WTOT], _AF.Identity)

            # sel is bf16: a 4-byte output operand would knock every DVE
            # reduce from 2x packed mode down to 1x (2x needs ALL non-scalar
            # operands 2-byte); the single bf16 rounding of sel is harmless.
            sel_t = sel_pool.tile([P, K], mybir.dt.bfloat16, tag="sel")
            # top selection product on GpSimd (DVE is the critical engine)
            prT_t = ub_pool.tile([P, WTOT], mybir.dt.bfloat16, tag="prT")
            nc.gpsimd.tensor_tensor(
                out=prT_t[:], in0=ub_t[:], in1=meta_t[:, 0:WTOT], op=_OP.mult
            )
            with nc.allow_low_precision("bf16 sel keeps DVE reduces in 2x mode"):
                nc.vector.tensor_reduce(
                    out=sel_t[:, 0:7],
                    in_=prT_t[:, 0 : 7 * 64].rearrange("p (k d) -> p k d", k=7),
                    axis=mybir.AxisListType.X,
                    op=_OP.add,
                )
                nc.vector.tensor_reduce(
                    out=sel_t[:, 7:8],
                    in_=prT_t[:, L7OFF : L7OFF + 128],
                    axis=mybir.AxisListType.X,
                    op=_OP.add,
                )
                nc.vector.tensor_reduce(
                    out=sel_t[:, 8:9],
                    in_=prT_t[:, L8OFF : L8OFF + 256],
                    axis=mybir.AxisListType.X,
                    op=_OP.add,
                )

                # deep product on DVE (bf16 2x), reduce 5 levels DVE + 2 ACT
                pr_t = scr_pool.tile([P, DEEPL * D], mybir.dt.bfloat16, tag="pr")
                fview = featb_t[:, blk * D : (blk + 1) * D]
                nc.vector.tensor_tensor(
                    out=pr_t[:].rearrange("p (k d) -> p k d", k=DEEPL),
                    in0=g_t[:].rearrange("p (k d) -> p k d", k=DEEPL),
                    in1=fview[:, None, :].to_broadcast([P, DEEPL, D]),
                    op=_OP.mult,
                )
                nc.vector.tensor_reduce(
                    out=sel_t[:, TOPL : TOPL + DVE_DEEP],
                    in_=pr_t[:, 0 : DVE_DEEP * D].rearrange(
                        "p (k d) -> p k d", k=DVE_DEEP
                    ),
                    axis=mybir.AxisListType.X,
                    op=_OP.add,
                )
                dump_t = scr_pool.tile([P, D], mybir.dt.bfloat16, tag="dump")
                for i in range(DVE_DEEP, DEEPL):
                    nc.scalar.activation(
                        dump_t[:],
                        pr_t[:, i * D : (i + 1) * D],
                        _AF.Identity,
                        accum_out=sel_t[:, TOPL + i : TOPL + i + 1],
                    )
                # masked-level-15 bias: sel += biasK (0 or -30)
                nc.gpsimd.tensor_tensor(
                    out=sel_t[:],
                    in0=sel_t[:],
                    in1=meta_t[:, WTOT : WTOT + K],
                    op=_OP.add,
                )
            # stash e^sel; ln/sum deferred past the block loop
            nc.scalar.activation(
                eall_t[:, blk * K : (blk + 1) * K], sel_t[:], _AF.Exp
            )

        # out[b] = sum_k ln(1 + e^sel) : one Ln (bias=1) + one 16-wide reduce
        spall_t = once_pool.tile([P, NBLK * K], mybir.dt.float32, tag="spall")
        nc.scalar.activation(spall_t[:], eall_t[:], _AF.Ln, bias=1.0)
        lnres_t = once_pool.tile([P, NBLK], mybir.dt.float32, tag="lnres")
        nc.vector.tensor_reduce(
            out=lnres_t[:],
            in_=spall_t[:].rearrange("p (blk k) -> p blk k", blk=NBLK),
            axis=mybir.AxisListType.X,
            op=_OP.add,
        )
        for blk in range(NBLK):
            nc.sync.dma_start(
                out_ap[blk * P : (blk + 1) * P, :], lnres_t[:, blk : blk + 1]
            )

    nc.compile()
    return nc


_PROGRAM_CACHE = {}


def _get_program():
    if "nc" not in _PROGRAM_CACHE:
        _PROGRAM_CACHE["nc"] = _build_program()
    return _PROGRAM_CACHE["nc"]


def _reset_device():
    # A previously-crashed kernel can leave an exec unit wedged; a
    # client-side axon reset clears it and is near-free otherwise.
    try:
        import ctypes

        lib = ctypes.CDLL("/opt/axon/libaxon_pjrt.so")
        lib.axon_reset.restype = ctypes.c_int64
        lib.axon_reset()
    except Exception:
        pass


def _prepare_inputs(features, targets, node_weights, path_nodes_map, path_directions_map):
    feat = np.asarray(features, dtype=np.float32)
    t = np.asarray(targets, dtype=np.int32).reshape(-1)
    nw = np.asarray(node_weights, dtype=np.float32)
    pn = np.asarray(path_nodes_map, dtype=np.int32)
    pd = np.asarray(path_directions_map, dtype=np.int32)

    diff = nw[:, :, 1] - nw[:, :, 0]                        # [N_INT, D]

    # deep per-class table, levels 9..15, sign+mask folded, bf16
    nodes_d = pn[:, TOPL:]
    dirs_d = pd[:, TOPL:]
    maskd = nodes_d != -1
    safed = np.where(maskd, nodes_d, 0)
    sgnd = np.where(maskd, 1 - 2 * dirs_d, 0).astype(np.float32)
    ptabd = (diff[safed] * sgnd[:, :, None]).reshape(NUM_CLASSES, DEEPL * D)
    ptabd = np.ascontiguousarray(ptabd.astype(BF16))

    # top-node column layout (within each feature chunk):
    #   level j<7 -> cols j*64 + (node - (2^j-1));  cols 448..511 zero
    #   level 7   -> cols 512 + (node-127);  level 8 -> cols 640 + (node-255)
    topcol = np.zeros(2 ** TOPL - 1, np.int64)
    for j in range(7):
        lo = 2 ** j - 1
        topcol[lo : 2 * lo + 1] = j * 64 + np.arange(lo + 1)
    topcol[127:255] = L7OFF + np.arange(128)
    topcol[255:511] = L8OFF + np.arange(256)

    dtopT = np.zeros((P, NCH, WTOT), np.float32)
    dtopT[:, :, topcol] = diff[: 2 ** TOPL - 1].reshape(-1, NCH, P).transpose(2, 1, 0)
    dtopT = np.ascontiguousarray(dtopT.reshape(P, NCH * WTOT).astype(BF16))

    # per-sample meta: +-1 one-hot over padded top cols | biasK
    n9 = pn[t, :TOPL]                                       # [B, 9] all valid
    d9 = pd[t, :TOPL]
    oh = np.zeros((B, WTOT), np.float32)
    oh[np.arange(B)[:, None], topcol[n9]] = 1 - 2 * d9
    biasK = np.zeros((B, K), np.float32)
    biasK[:, K - 1] = np.where(pn[t, K - 1] == -1, -30.0, 0.0)
    meta = np.ascontiguousarray(
        np.concatenate([oh, biasK], axis=1).astype(BF16)     # [B, WTOT+K]
    )

    per_core = []
    for i in range(N_CORES):
        sl = slice(i * BL, (i + 1) * BL)
        fc = feat[sl]
        tc_ = t[sl]
        featb = np.ascontiguousarray(
            fc.reshape(NBLK, P, D).transpose(1, 0, 2).reshape(P, NBLK * D).astype(BF16)
        )
        featT = np.ascontiguousarray(
            fc.reshape(BL, NCH, P).transpose(2, 1, 0).reshape(P, NCH * BL).astype(BF16)
        )
        idx = np.ascontiguousarray(tc_.reshape(NBLK, P).T.astype(np.int32))
        per_core.append(
            {
                "ptabd": ptabd,
                "dtopT": dtopT,
                "idx": idx,
                "featb": featb,
                "featT": featT,
                "meta": meta[sl],
            }
        )
    return per_core


def kernel(features, targets, node_weights, path_nodes_map, path_directions_map):
    in_maps = _prepare_inputs(
        features, targets, node_weights, path_nodes_map, path_directions_map
    )
    _reset_device()
    nc = _get_program()
    res = bass_utils.run_bass_kernel_spmd(nc, in_maps, core_ids=list(range(N_CORES)))
    out = np.concatenate([res.results[i]["out"].reshape(-1) for i in range(N_CORES)])
    return out.astype(np.float32)
